# revision 3
# baseline (speedup 1.0000x reference)
"""Phi3 fast attention (B=2, S=2048, H=3072, 32 q heads / 8 kv heads, hd=96)
on 8 Trainium2 NeuronCores.

Sharding: tensor-parallel over heads. Core r owns q heads 4r..4r+3 and kv
head r. Each core computes its slice of the fused QKV projection, RoPE,
causal flash-style attention for its 4 q heads, then the parts of the
attention output are AllGathered (per batch) and each core computes a
384-row slice of the output projection. The host concatenates the 8
output slices.

All matmuls run in float32r (TF32-like: 11-bit mantissa, full fp32
exponent) at full PE rate; softmax/normalization in fp32.

Device-side layouts (feature dims on SBUF partitions, tokens on free dim):
 - padded head dim: real dim j at partition (j if j < 48 else j + 16);
   partitions 48..63 and 112..127 are zero pads. This makes the RoPE
   rotate-half a partition rotation by 64, done with two shifted copies.
 - x_t:    hidden states transposed [H=3072, T=4096] (token t = b*2048+s)
 - qkv out per feature block fb (q0,q1,q2,q3,k,v): [128, T]
 - scores computed transposed [keys, q-tokens]; softmax denominator via an
   appended ones-column in the transposed V (row 96 of the PV accumulator).
"""

import numpy as np

import concourse.bass as bass
import concourse.mybir as mybir
import concourse.tile as tile
import concourse.bacc as bacc
from concourse.bass_utils import run_bass_kernel_spmd

N_CORES = 8
B, S, H = 2, 2048, 3072
NH, NKV, HD = 32, 8, 96
T = B * S
ROPE_BASE = 10000.0
KC = H // 128          # 24 contraction chunks
TBQ = 256              # qkv token block
NTQ = T // TBQ         # 16
TBA = 512              # attention / o_proj token block
NKB = S // 128         # 16 key blocks per batch
SCALE = 1.0 / float(np.sqrt(HD))

F32 = mybir.dt.float32
F32R = mybir.dt.float32r

_NC_CACHE = {}


def _round_tf32(x: np.ndarray) -> np.ndarray:
    u = np.ascontiguousarray(x, dtype=np.float32).view(np.uint32)
    r = (u + 0x7FF + ((u >> 12) & 1)) & 0xFFFFF000
    return r.astype(np.uint32).view(np.float32)


def _pad_head(w96: np.ndarray) -> np.ndarray:
    """[96, H] head rows -> [128, H] padded layout."""
    out = np.zeros((128, w96.shape[1]), dtype=w96.dtype)
    out[0:48] = w96[0:48]
    out[64:112] = w96[48:96]
    return out


def _build_nc():
    if "nc" in _NC_CACHE:
        return _NC_CACHE["nc"]

    nc = bacc.Bacc("TRN2", target_bir_lowering=False, debug=False,
                   num_devices=N_CORES)

    x_t = nc.dram_tensor("x_t", [H, T], F32R, kind="ExternalInput").ap()
    w_s = nc.dram_tensor("w_s", [6, 128, KC, 128], F32R,
                         kind="ExternalInput").ap()
    wo_s = nc.dram_tensor("wo_s", [3, 128, KC, 128], F32R,
                          kind="ExternalInput").ap()
    cos_d = nc.dram_tensor("cos_d", [128, T], F32, kind="ExternalInput").ap()
    sin_d = nc.dram_tensor("sin_d", [128, T], F32, kind="ExternalInput").ap()
    mask_d = nc.dram_tensor("mask_d", [128, 896], F32,
                            kind="ExternalInput").ap()
    ident_d = nc.dram_tensor("ident_d", [128, 128], F32,
                             kind="ExternalInput").ap()
    out_d = nc.dram_tensor("out", [384, T], F32, kind="ExternalOutput").ap()

    rg = [list(range(N_CORES))]

    with tile.TileContext(nc) as tc:
        with (
            tc.tile_pool(name="const", bufs=1) as constp,
            tc.tile_pool(name="qk", bufs=1) as qkp,
            tc.tile_pool(name="dram", bufs=1, space="DRAM") as dramp,
        ):
            mask_t = constp.tile([128, 896], F32, name="mask_t")
            nc.sync.dma_start(out=mask_t, in_=mask_d)
            ident_t = constp.tile([128, 128], F32, name="ident_t")
            nc.sync.dma_start(out=ident_t, in_=ident_d)

            # q0..q3 then k — rope'd, fp32r, padded head layout
            qf = [qkp.tile([128, T], F32R, name=f"qf{i}", tag=f"qf{i}")
                  for i in range(5)]
            # transposed V per batch: [keys=128, 16 blocks x 97] with a ones
            # column at 97k+96 (softmax denominator accumulator row)
            vk = [qkp.tile([128, NKB * 97], F32R, name=f"vk{b}", tag=f"vk{b}")
                  for b in range(B)]
            for b in range(B):
                for kb in range(NKB):
                    nc.vector.memset(
                        vk[b][:, 97 * kb + 96:97 * kb + 97].bitcast(F32), 1.0)

            ag_in = [dramp.tile([384, S], F32R, name=f"agin{b}",
                                tag=f"agin{b}") for b in range(B)]
            ag_out = [dramp.tile([N_CORES * 384, S], F32R,
                                 addr_space="Shared", name=f"agout{b}",
                                 tag=f"agout{b}") for b in range(B)]

            with (
                tc.tile_pool(name="pa", bufs=1) as pa,
                tc.tile_pool(name="pb", bufs=1) as pb,
                tc.tile_pool(name="psA", bufs=2, space="PSUM") as psA,
                tc.tile_pool(name="psT", bufs=1, space="PSUM") as psT,
                tc.tile_pool(name="psB", bufs=2, space="PSUM") as psB,
            ):
                # ---------------- Phase A: QKV projection + RoPE ----------
                # two fb-groups so only 3 w tiles are resident; group 0
                # holds (q0, k, v) so head-0 attention can start early.
                for g, fbs in enumerate([[0, 4, 5], [1, 2, 3]]):
                    wt = {}
                    for slot, fb in enumerate(fbs):
                        w_tile = pa.tile([128, KC * 128], F32R,
                                         tag=f"w{slot}", name=f"w_{g}_{fb}")
                        nc.sync.dma_start(
                            out=w_tile,
                            in_=w_s[fb].rearrange("p k j -> p (k j)"))
                        wt[fb] = w_tile
                    for tb in range(NTQ):
                        ts = slice(tb * TBQ, (tb + 1) * TBQ)
                        xts = []
                        for k in range(KC):
                            xt = pa.tile([128, TBQ], F32R, tag=f"x{k}",
                                         name=f"x_{g}_{tb}_{k}")
                            nc.sync.dma_start(
                                out=xt, in_=x_t[k * 128:(k + 1) * 128, ts])
                            xts.append(xt)
                        cos_tb = pa.tile([128, TBQ], F32, tag="cos", bufs=2,
                                         name=f"cos_{g}_{tb}")
                        sin_tb = pa.tile([128, TBQ], F32, tag="sin", bufs=2,
                                         name=f"sin_{g}_{tb}")
                        nc.sync.dma_start(out=cos_tb, in_=cos_d[:, ts])
                        nc.sync.dma_start(out=sin_tb, in_=sin_d[:, ts])
                        for fb in fbs:
                            ps = psA.tile([128, TBQ], F32, tag="qkv",
                                          name=f"ps_{g}_{tb}_{fb}")
                            for k in range(KC):
                                nc.tensor.matmul(
                                    ps, wt[fb][:, k * 128:(k + 1) * 128],
                                    xts[k], start=(k == 0),
                                    stop=(k == KC - 1))
                            if fb == 5:
                                # V: transpose to [keys, d] layout
                                vst = pa.tile([128, TBQ], F32, tag="vst",
                                              bufs=2, name=f"vst_{tb}")
                                nc.vector.tensor_copy(vst, ps)
                                b = tb // (NTQ // B)
                                for i in range(TBQ // 128):
                                    kb = (tb % (NTQ // B)) * (TBQ // 128) + i
                                    pt = psT.tile([128, 128], F32, tag="tr",
                                                  name=f"pt_{tb}_{i}")
                                    nc.tensor.transpose(
                                        pt, vst[:, i * 128:(i + 1) * 128],
                                        ident_t)
                                    c0 = 97 * kb
                                    nc.vector.tensor_copy(
                                        vk[b][:, c0:c0 + 48], pt[:, 0:48])
                                    nc.vector.tensor_copy(
                                        vk[b][:, c0 + 48:c0 + 96],
                                        pt[:, 64:112])
                            else:
                                dst = qf[fb] if fb < 4 else qf[4]
                                rot = pa.tile([128, TBQ], F32, tag="rot",
                                              bufs=2, name=f"rot_{g}_{tb}_{fb}")
                                nc.vector.tensor_copy(rot[0:64, :],
                                                      ps[64:128, :])
                                nc.vector.tensor_copy(rot[64:128, :],
                                                      ps[0:64, :])
                                nc.vector.tensor_mul(rot, rot, sin_tb)
                                cq = pa.tile([128, TBQ], F32, tag="cq",
                                             bufs=2, name=f"cq_{g}_{tb}_{fb}")
                                nc.vector.tensor_mul(cq, ps, cos_tb)
                                nc.vector.tensor_add(dst[:, ts], rot, cq)

                # ---------------- Phase B: causal attention ---------------
                for b in range(B):
                    for h in range(4):
                        for qb in range(S // TBA):
                            t0 = b * S + qb * TBA
                            qs = slice(t0, t0 + TBA)
                            pv = psB.tile([128, TBA], F32, tag="pv",
                                          name=f"pv_{b}_{h}_{qb}")
                            nkb = (qb + 1) * (TBA // 128)
                            for kb in range(nkb):
                                sc = psB.tile([128, TBA], F32, tag="sc",
                                              name=f"sc_{b}_{h}_{qb}_{kb}")
                                nc.tensor.matmul(
                                    sc,
                                    qf[4][:, b * S + kb * 128:
                                          b * S + (kb + 1) * 128],
                                    qf[h][:, qs], start=True, stop=True)
                                diag0 = qb * (TBA // 128)
                                pr = pb.tile([128, TBA], F32R, tag="pr",
                                             bufs=3,
                                             name=f"pr_{b}_{h}_{qb}_{kb}")
                                if kb >= diag0:
                                    prf = pb.tile(
                                        [128, TBA], F32, tag="prf", bufs=2,
                                        name=f"prf_{b}_{h}_{qb}_{kb}")
                                    nc.scalar.activation(
                                        prf, sc,
                                        mybir.ActivationFunctionType.Exp,
                                        scale=SCALE)
                                    off = (kb - diag0) * 128
                                    nc.vector.tensor_mul(
                                        pr, prf,
                                        mask_t[:, 384 - off:896 - off])
                                else:
                                    nc.scalar.activation(
                                        pr, sc,
                                        mybir.ActivationFunctionType.Exp,
                                        scale=SCALE)
                                nc.tensor.matmul(
                                    pv[0:97, :],
                                    vk[b][:, 97 * kb:97 * kb + 97], pr,
                                    start=(kb == 0), stop=(kb == nkb - 1))
                            # normalize: attn = pv[0:96] / pv[96]
                            l_r = pb.tile([1, TBA], F32, tag="lr", bufs=2,
                                          name=f"lr_{b}_{h}_{qb}")
                            nc.vector.reciprocal(l_r, pv[96:97, :])
                            nt = pb.tile([1, TBA], F32, tag="nt", bufs=2,
                                         name=f"nt_{b}_{h}_{qb}")
                            # one Newton step: r <- r * (2 - l * r)
                            nc.vector.tensor_mul(nt, pv[96:97, :], l_r)
                            nc.vector.tensor_scalar(
                                nt, nt, -1.0, 2.0,
                                op0=mybir.AluOpType.mult,
                                op1=mybir.AluOpType.add)
                            nc.vector.tensor_mul(l_r, l_r, nt)
                            bc = pb.tile([96, TBA], F32, tag="bc", bufs=2,
                                         name=f"bc_{b}_{h}_{qb}")
                            nc.gpsimd.partition_broadcast(bc, l_r)
                            asb = pb.tile([96, TBA], F32R, tag="asb", bufs=3,
                                          name=f"asb_{b}_{h}_{qb}")
                            nc.vector.tensor_mul(asb, pv[0:96, :], bc)
                            nc.sync.dma_start(
                                out=ag_in[b][96 * h:96 * (h + 1),
                                             qb * TBA:(qb + 1) * TBA],
                                in_=asb)
                    nc.gpsimd.collective_compute(
                        "AllGather", mybir.AluOpType.bypass,
                        replica_groups=rg,
                        ins=[ag_in[b].opt()], outs=[ag_out[b].opt()])

            # ---------------- Phase C: output projection -----------------
            with (
                tc.tile_pool(name="pc", bufs=1) as pc,
                tc.tile_pool(name="psC", bufs=2, space="PSUM") as psC,
            ):
                wo = []
                for jb in range(3):
                    wo_tile = pc.tile([128, KC * 128], F32R, tag=f"wo{jb}",
                                      name=f"wo_{jb}")
                    nc.sync.dma_start(
                        out=wo_tile, in_=wo_s[jb].rearrange("p k j -> p (k j)"))
                    wo.append(wo_tile)
                for b in range(B):
                    for tb in range(S // TBA):
                        cs = slice(tb * TBA, (tb + 1) * TBA)
                        ats = []
                        for k in range(KC):
                            at = pc.tile([128, TBA], F32R, tag=f"a{k}",
                                         name=f"at_{b}_{tb}_{k}")
                            nc.sync.dma_start(
                                out=at,
                                in_=ag_out[b][k * 128:(k + 1) * 128, cs])
                            ats.append(at)
                        for jb in range(3):
                            ps = psC.tile([128, TBA], F32, tag="op",
                                          name=f"ops_{b}_{tb}_{jb}")
                            for k in range(KC):
                                nc.tensor.matmul(
                                    ps, wo[jb][:, k * 128:(k + 1) * 128],
                                    ats[k], start=(k == 0),
                                    stop=(k == KC - 1))
                            osb = pc.tile([128, TBA], F32, tag="ost", bufs=3,
                                          name=f"osb_{b}_{tb}_{jb}")
                            nc.vector.tensor_copy(osb, ps)
                            nc.sync.dma_start(
                                out=out_d[jb * 128:(jb + 1) * 128,
                                          b * S + tb * TBA:
                                          b * S + (tb + 1) * TBA],
                                in_=osb)

    nc.compile()
    _NC_CACHE["nc"] = nc
    return nc


def _prep_inputs(hidden_states, position_ids, w_qkv, w_o):
    hidden_states = np.asarray(hidden_states, dtype=np.float32)
    position_ids = np.asarray(position_ids)
    w_qkv = np.asarray(w_qkv, dtype=np.float32)
    w_o = np.asarray(w_o, dtype=np.float32)

    x_t = _round_tf32(hidden_states.reshape(T, H).T)

    # rope trig in padded-partition layout
    pos = position_ids.reshape(T).astype(np.float64)
    inv_freq = 1.0 / (ROPE_BASE ** (np.arange(0, HD, 2, dtype=np.float64)
                                    / HD))  # [48]
    freqs = pos[None, :] * inv_freq[:, None]          # [48, T]
    cos48, sin48 = np.cos(freqs), np.sin(freqs)
    cos_d = np.zeros((128, T), dtype=np.float32)
    sin_d = np.zeros((128, T), dtype=np.float32)
    cos_d[0:48] = cos48
    cos_d[64:112] = cos48
    sin_d[0:48] = -sin48
    sin_d[64:112] = sin48

    # causal mask staircase: ext[kk, c] = 1 if c >= kk + 384
    cc = np.arange(896)[None, :]
    kk = np.arange(128)[:, None]
    mask_ext = (cc >= kk + 384).astype(np.float32)

    ident = np.eye(128, dtype=np.float32)

    common = {"x_t": x_t, "cos_d": cos_d, "sin_d": sin_d,
              "mask_d": mask_ext, "ident_d": ident}

    in_maps = []
    for r in range(N_CORES):
        w_big = np.zeros((6, 128, H), dtype=np.float32)
        for fb in range(4):
            qh = 4 * r + fb
            w_big[fb] = _pad_head(w_qkv[96 * qh:96 * (qh + 1), :])
        w_big[4] = _pad_head(w_qkv[NH * HD + 96 * r: NH * HD + 96 * (r + 1)])
        w_big[5] = _pad_head(
            w_qkv[NH * HD + NKV * HD + 96 * r: NH * HD + NKV * HD + 96 * (r + 1)])
        # [6, 128(j), 24(k), 128(p)] -> [6, 128(p), 24(k), 128(j)]
        w_dev = np.ascontiguousarray(
            w_big.reshape(6, 128, KC, 128).transpose(0, 3, 2, 1))
        w_dev = _round_tf32(w_dev)

        wo_shard = w_o[384 * r:384 * (r + 1), :]   # [384, 3072]
        wo_dev = np.ascontiguousarray(
            wo_shard.reshape(3, 128, KC, 128).transpose(0, 3, 2, 1))
        wo_dev = _round_tf32(wo_dev)

        in_maps.append({**common, "w_s": w_dev, "wo_s": wo_dev})
    return in_maps


def kernel(hidden_states, position_ids, w_qkv, w_o):
    in_maps = _prep_inputs(hidden_states, position_ids, w_qkv, w_o)
    nc = _build_nc()
    res = run_bass_kernel_spmd(nc, in_maps, core_ids=list(range(N_CORES)))
    big = np.concatenate([res.results[r]["out"] for r in range(N_CORES)],
                         axis=0)                      # [3072, 4096]
    out = big.reshape(H, B, S).transpose(1, 2, 0)     # [B, S, H]
    return np.ascontiguousarray(out, dtype=np.float32)


# revision 4
# speedup vs baseline: 32.5933x; 32.5933x over previous
"""Phi3 fast attention (B=2, S=2048, H=3072, 32 q heads / 8 kv heads, hd=96)
on 8 Trainium2 NeuronCores.

Sharding: tensor-parallel over heads. Core r owns q heads 4r..4r+3 and kv
head r. Each core computes its slice of the fused QKV projection, RoPE,
causal flash-style attention for its 4 q heads, then the parts of the
attention output are AllGathered (per batch) and each core computes a
384-row slice of the output projection. The host concatenates the 8
output slices.

All matmuls run in float32r (TF32-like: 11-bit mantissa, full fp32
exponent) at full PE rate; softmax/normalization in fp32.

Device-side layouts (feature dims on SBUF partitions, tokens on free dim):
 - padded head dim: real dim j at partition (j if j < 48 else j + 16);
   partitions 48..63 and 112..127 are zero pads. This makes the RoPE
   rotate-half a partition rotation by 64, done with two shifted copies.
 - x_t:    hidden states transposed [H=3072, T=4096] (token t = b*2048+s)
 - qkv out per feature block fb (q0,q1,q2,q3,k,v): [128, T]
 - scores computed transposed [keys, q-tokens]; softmax denominator via an
   appended ones-column in the transposed V (row 96 of the PV accumulator).
"""

import numpy as np

import concourse.bass as bass
import concourse.mybir as mybir
import concourse.tile as tile
import concourse.bacc as bacc
from concourse.bass_utils import run_bass_kernel_spmd

N_CORES = 8
B, S, H = 2, 2048, 3072
NH, NKV, HD = 32, 8, 96
T = B * S
ROPE_BASE = 10000.0
KC = H // 128          # 24 contraction chunks
TBQ = 256              # qkv token block
NTQ = T // TBQ         # 16
TBA = 512              # attention / o_proj token block
NKB = S // 128         # 16 key blocks per batch
SCALE = 1.0 / float(np.sqrt(HD))

F32 = mybir.dt.float32
F32R = mybir.dt.float32r

_NC_CACHE = {}


def _round_tf32(x: np.ndarray) -> np.ndarray:
    u = np.ascontiguousarray(x, dtype=np.float32).view(np.uint32)
    r = (u + 0x7FF + ((u >> 12) & 1)) & 0xFFFFF000
    return r.astype(np.uint32).view(np.float32)


def _pad_head(w96: np.ndarray) -> np.ndarray:
    """[96, H] head rows -> [128, H] padded layout."""
    out = np.zeros((128, w96.shape[1]), dtype=w96.dtype)
    out[0:48] = w96[0:48]
    out[64:112] = w96[48:96]
    return out


def _build_nc(repeat: int = 1):
    key = ("nc", repeat)
    if key in _NC_CACHE:
        return _NC_CACHE[key]

    nc = bacc.Bacc("TRN2", target_bir_lowering=False, debug=False,
                   num_devices=N_CORES)

    x_t = nc.dram_tensor("x_t", [H, T], F32R, kind="ExternalInput").ap()
    w_s = nc.dram_tensor("w_s", [6, 128, KC, 128], F32R,
                         kind="ExternalInput").ap()
    wo_s = nc.dram_tensor("wo_s", [3, 128, KC, 128], F32R,
                          kind="ExternalInput").ap()
    cos_d = nc.dram_tensor("cos_d", [128, T], F32, kind="ExternalInput").ap()
    sin_d = nc.dram_tensor("sin_d", [128, T], F32, kind="ExternalInput").ap()
    mask_d = nc.dram_tensor("mask_d", [128, 896], F32,
                            kind="ExternalInput").ap()
    ident_d = nc.dram_tensor("ident_d", [128, 128], F32,
                             kind="ExternalInput").ap()
    out_d = nc.dram_tensor("out", [384, T], F32, kind="ExternalOutput").ap()

    with tile.TileContext(nc) as tc:
        for rep in range(repeat):
            if rep:
                tc.strict_bb_all_engine_barrier()
            _emit_program(nc, tc, x_t, w_s, wo_s, cos_d, sin_d, mask_d,
                          ident_d, out_d, rep)

    nc.compile()
    _NC_CACHE[key] = nc
    return nc


def _emit_program(nc, tc, x_t, w_s, wo_s, cos_d, sin_d, mask_d, ident_d,
                  out_d, rep):
    rg = [list(range(N_CORES))]
    if True:
        with (
            tc.tile_pool(name="const", bufs=1) as constp,
            tc.tile_pool(name="qk", bufs=1) as qkp,
            tc.tile_pool(name="dram", bufs=1, space="DRAM") as dramp,
        ):
            mask_t = constp.tile([128, 896], F32, name="mask_t")
            nc.sync.dma_start(out=mask_t, in_=mask_d)
            ident_t = constp.tile([128, 128], F32, name="ident_t")
            nc.sync.dma_start(out=ident_t, in_=ident_d)

            # q0..q3 then k — rope'd, fp32r, padded head layout
            qf = [qkp.tile([128, T], F32R, name=f"qf{i}", tag=f"qf{i}")
                  for i in range(5)]
            # transposed V per batch: [keys=128, 16 blocks x 97] with a ones
            # column at 97k+96 (softmax denominator accumulator row)
            vk = [qkp.tile([128, NKB * 97], F32R, name=f"vk{b}", tag=f"vk{b}")
                  for b in range(B)]
            for b in range(B):
                for kb in range(NKB):
                    nc.vector.memset(
                        vk[b][:, 97 * kb + 96:97 * kb + 97].bitcast(F32), 1.0)

            ag_in = [dramp.tile([384, S], F32R, name=f"agin{b}",
                                tag=f"agin{b}") for b in range(B)]
            ag_out = [dramp.tile([N_CORES * 384, S], F32R,
                                 addr_space="Shared", name=f"agout{b}",
                                 tag=f"agout{b}") for b in range(B)]

            with (
                tc.tile_pool(name="pa", bufs=1) as pa,
                tc.tile_pool(name="pb", bufs=1) as pb,
                tc.tile_pool(name="psA", bufs=2, space="PSUM") as psA,
                tc.tile_pool(name="psT", bufs=1, space="PSUM") as psT,
                tc.tile_pool(name="psB", bufs=2, space="PSUM") as psB,
            ):
                # ---------------- Phase A: QKV projection + RoPE ----------
                # two fb-groups so only 3 w tiles are resident; group 0
                # holds (q0, k, v) so head-0 attention can start early.
                for g, fbs in enumerate([[0, 4, 5], [1, 2, 3]]):
                    wt = {}
                    for slot, fb in enumerate(fbs):
                        w_tile = pa.tile([128, KC * 128], F32R,
                                         tag=f"w{slot}", name=f"w_{g}_{fb}")
                        nc.sync.dma_start(
                            out=w_tile,
                            in_=w_s[fb].rearrange("p k j -> p (k j)"))
                        wt[fb] = w_tile
                    for tb in range(NTQ):
                        ts = slice(tb * TBQ, (tb + 1) * TBQ)
                        xts = []
                        for k in range(KC):
                            xt = pa.tile([128, TBQ], F32R, tag=f"x{k}",
                                         name=f"x_{g}_{tb}_{k}")
                            nc.sync.dma_start(
                                out=xt, in_=x_t[k * 128:(k + 1) * 128, ts])
                            xts.append(xt)
                        cos_tb = pa.tile([128, TBQ], F32, tag="cos", bufs=2,
                                         name=f"cos_{g}_{tb}")
                        sin_tb = pa.tile([128, TBQ], F32, tag="sin", bufs=2,
                                         name=f"sin_{g}_{tb}")
                        nc.sync.dma_start(out=cos_tb, in_=cos_d[:, ts])
                        nc.sync.dma_start(out=sin_tb, in_=sin_d[:, ts])
                        for fb in fbs:
                            ps = psA.tile([128, TBQ], F32, tag="qkv",
                                          name=f"ps_{g}_{tb}_{fb}")
                            for k in range(KC):
                                nc.tensor.matmul(
                                    ps, wt[fb][:, k * 128:(k + 1) * 128],
                                    xts[k], start=(k == 0),
                                    stop=(k == KC - 1))
                            if fb == 5:
                                # V: transpose to [keys, d] layout
                                vst = pa.tile([128, TBQ], F32, tag="vst",
                                              bufs=2, name=f"vst_{tb}")
                                nc.vector.tensor_copy(vst, ps)
                                b = tb // (NTQ // B)
                                for i in range(TBQ // 128):
                                    kb = (tb % (NTQ // B)) * (TBQ // 128) + i
                                    pt = psT.tile([128, 128], F32, tag="tr",
                                                  name=f"pt_{tb}_{i}")
                                    nc.tensor.transpose(
                                        pt, vst[:, i * 128:(i + 1) * 128],
                                        ident_t)
                                    c0 = 97 * kb
                                    nc.vector.tensor_copy(
                                        vk[b][:, c0:c0 + 48], pt[:, 0:48])
                                    nc.vector.tensor_copy(
                                        vk[b][:, c0 + 48:c0 + 96],
                                        pt[:, 64:112])
                            else:
                                dst = qf[fb] if fb < 4 else qf[4]
                                rot = pa.tile([128, TBQ], F32, tag="rot",
                                              bufs=2, name=f"rot_{g}_{tb}_{fb}")
                                nc.vector.tensor_copy(rot[0:64, :],
                                                      ps[64:128, :])
                                nc.vector.tensor_copy(rot[64:128, :],
                                                      ps[0:64, :])
                                nc.vector.tensor_mul(rot, rot, sin_tb)
                                cq = pa.tile([128, TBQ], F32, tag="cq",
                                             bufs=2, name=f"cq_{g}_{tb}_{fb}")
                                nc.vector.tensor_mul(cq, ps, cos_tb)
                                nc.vector.tensor_add(dst[:, ts], rot, cq)

                # ---------------- Phase B: causal attention ---------------
                for b in range(B):
                    for h in range(4):
                        for qb in range(S // TBA):
                            t0 = b * S + qb * TBA
                            qs = slice(t0, t0 + TBA)
                            pv = psB.tile([128, TBA], F32, tag="pv",
                                          name=f"pv_{b}_{h}_{qb}")
                            nkb = (qb + 1) * (TBA // 128)
                            for kb in range(nkb):
                                sc = psB.tile([128, TBA], F32, tag="sc",
                                              name=f"sc_{b}_{h}_{qb}_{kb}")
                                nc.tensor.matmul(
                                    sc,
                                    qf[4][:, b * S + kb * 128:
                                          b * S + (kb + 1) * 128],
                                    qf[h][:, qs], start=True, stop=True)
                                diag0 = qb * (TBA // 128)
                                pr = pb.tile([128, TBA], F32R, tag="pr",
                                             bufs=3,
                                             name=f"pr_{b}_{h}_{qb}_{kb}")
                                if kb >= diag0:
                                    prf = pb.tile(
                                        [128, TBA], F32, tag="prf", bufs=2,
                                        name=f"prf_{b}_{h}_{qb}_{kb}")
                                    nc.scalar.activation(
                                        prf, sc,
                                        mybir.ActivationFunctionType.Exp,
                                        scale=SCALE)
                                    off = (kb - diag0) * 128
                                    nc.vector.tensor_mul(
                                        pr, prf,
                                        mask_t[:, 384 - off:896 - off])
                                else:
                                    nc.scalar.activation(
                                        pr, sc,
                                        mybir.ActivationFunctionType.Exp,
                                        scale=SCALE)
                                nc.tensor.matmul(
                                    pv[0:97, :],
                                    vk[b][:, 97 * kb:97 * kb + 97], pr,
                                    start=(kb == 0), stop=(kb == nkb - 1))
                            # normalize: attn = pv[0:96] / pv[96]
                            l_r = pb.tile([1, TBA], F32, tag="lr", bufs=2,
                                          name=f"lr_{b}_{h}_{qb}")
                            nc.vector.reciprocal(l_r, pv[96:97, :])
                            nt = pb.tile([1, TBA], F32, tag="nt", bufs=2,
                                         name=f"nt_{b}_{h}_{qb}")
                            # one Newton step: r <- r * (2 - l * r)
                            nc.vector.tensor_mul(nt, pv[96:97, :], l_r)
                            nc.vector.tensor_scalar(
                                nt, nt, -1.0, 2.0,
                                op0=mybir.AluOpType.mult,
                                op1=mybir.AluOpType.add)
                            nc.vector.tensor_mul(l_r, l_r, nt)
                            bc = pb.tile([96, TBA], F32, tag="bc", bufs=2,
                                         name=f"bc_{b}_{h}_{qb}")
                            nc.gpsimd.partition_broadcast(bc, l_r)
                            asb = pb.tile([96, TBA], F32R, tag="asb", bufs=3,
                                          name=f"asb_{b}_{h}_{qb}")
                            nc.vector.tensor_mul(asb, pv[0:96, :], bc)
                            nc.sync.dma_start(
                                out=ag_in[b][96 * h:96 * (h + 1),
                                             qb * TBA:(qb + 1) * TBA],
                                in_=asb)
                    nc.gpsimd.collective_compute(
                        "AllGather", mybir.AluOpType.bypass,
                        replica_groups=rg,
                        ins=[ag_in[b].opt()], outs=[ag_out[b].opt()])

            # ---------------- Phase C: output projection -----------------
            with (
                tc.tile_pool(name="pc", bufs=1) as pc,
                tc.tile_pool(name="psC", bufs=2, space="PSUM") as psC,
            ):
                wo = []
                for jb in range(3):
                    wo_tile = pc.tile([128, KC * 128], F32R, tag=f"wo{jb}",
                                      name=f"wo_{jb}")
                    nc.sync.dma_start(
                        out=wo_tile, in_=wo_s[jb].rearrange("p k j -> p (k j)"))
                    wo.append(wo_tile)
                for b in range(B):
                    for tb in range(S // TBA):
                        cs = slice(tb * TBA, (tb + 1) * TBA)
                        ats = []
                        for k in range(KC):
                            at = pc.tile([128, TBA], F32R, tag=f"a{k}",
                                         name=f"at_{b}_{tb}_{k}")
                            nc.sync.dma_start(
                                out=at,
                                in_=ag_out[b][k * 128:(k + 1) * 128, cs])
                            ats.append(at)
                        for jb in range(3):
                            ps = psC.tile([128, TBA], F32, tag="op",
                                          name=f"ops_{b}_{tb}_{jb}")
                            for k in range(KC):
                                nc.tensor.matmul(
                                    ps, wo[jb][:, k * 128:(k + 1) * 128],
                                    ats[k], start=(k == 0),
                                    stop=(k == KC - 1))
                            osb = pc.tile([128, TBA], F32, tag="ost", bufs=3,
                                          name=f"osb_{b}_{tb}_{jb}")
                            nc.vector.tensor_copy(osb, ps)
                            nc.sync.dma_start(
                                out=out_d[jb * 128:(jb + 1) * 128,
                                          b * S + tb * TBA:
                                          b * S + (tb + 1) * TBA],
                                in_=osb)


def _prep_inputs(hidden_states, position_ids, w_qkv, w_o):
    hidden_states = np.asarray(hidden_states, dtype=np.float32)
    position_ids = np.asarray(position_ids)
    w_qkv = np.asarray(w_qkv, dtype=np.float32)
    w_o = np.asarray(w_o, dtype=np.float32)

    x_t = _round_tf32(hidden_states.reshape(T, H).T)

    # rope trig in padded-partition layout
    pos = position_ids.reshape(T).astype(np.float64)
    inv_freq = 1.0 / (ROPE_BASE ** (np.arange(0, HD, 2, dtype=np.float64)
                                    / HD))  # [48]
    freqs = pos[None, :] * inv_freq[:, None]          # [48, T]
    cos48, sin48 = np.cos(freqs), np.sin(freqs)
    cos_d = np.zeros((128, T), dtype=np.float32)
    sin_d = np.zeros((128, T), dtype=np.float32)
    cos_d[0:48] = cos48
    cos_d[64:112] = cos48
    sin_d[0:48] = -sin48
    sin_d[64:112] = sin48

    # causal mask staircase: ext[kk, c] = 1 if c >= kk + 384
    cc = np.arange(896)[None, :]
    kk = np.arange(128)[:, None]
    mask_ext = (cc >= kk + 384).astype(np.float32)

    ident = np.eye(128, dtype=np.float32)

    common = {"x_t": x_t, "cos_d": cos_d, "sin_d": sin_d,
              "mask_d": mask_ext, "ident_d": ident}

    in_maps = []
    for r in range(N_CORES):
        w_big = np.zeros((6, 128, H), dtype=np.float32)
        for fb in range(4):
            qh = 4 * r + fb
            w_big[fb] = _pad_head(w_qkv[96 * qh:96 * (qh + 1), :])
        w_big[4] = _pad_head(w_qkv[NH * HD + 96 * r: NH * HD + 96 * (r + 1)])
        w_big[5] = _pad_head(
            w_qkv[NH * HD + NKV * HD + 96 * r: NH * HD + NKV * HD + 96 * (r + 1)])
        # [6, 128(j), 24(k), 128(p)] -> [6, 128(p), 24(k), 128(j)]
        w_dev = np.ascontiguousarray(
            w_big.reshape(6, 128, KC, 128).transpose(0, 3, 2, 1))
        w_dev = _round_tf32(w_dev)

        wo_shard = w_o[384 * r:384 * (r + 1), :]   # [384, 3072]
        wo_dev = np.ascontiguousarray(
            wo_shard.reshape(3, 128, KC, 128).transpose(0, 3, 2, 1))
        wo_dev = _round_tf32(wo_dev)

        in_maps.append({**common, "w_s": w_dev, "wo_s": wo_dev})
    return in_maps


def kernel(hidden_states, position_ids, w_qkv, w_o):
    in_maps = _prep_inputs(hidden_states, position_ids, w_qkv, w_o)
    nc = _build_nc()
    res = run_bass_kernel_spmd(nc, in_maps, core_ids=list(range(N_CORES)))
    big = np.concatenate([res.results[r]["out"] for r in range(N_CORES)],
                         axis=0)                      # [3072, 4096]
    out = big.reshape(H, B, S).transpose(1, 2, 0)     # [B, S, H]
    return np.ascontiguousarray(out, dtype=np.float32)


# revision 14
# speedup vs baseline: 290.5342x; 8.9139x over previous
"""Phi3 fast attention (B=2, S=2048, H=3072, 32 q heads / 8 kv heads, hd=96)
on 8 Trainium2 NeuronCores.

Sharding: tensor-parallel over heads. Core r owns q heads 4r..4r+3 and kv
head r. Each core computes its slice of the fused QKV projection, RoPE,
causal flash-style attention for its 4 q heads, then the parts of the
attention output are AllGathered (per batch) and each core computes a
384-row slice of the output projection. The host concatenates the slices.

All matmuls run in float32r (TF32-like: 11-bit mantissa, full fp32
exponent) at full PE rate; softmax/normalization in fp32.

Structure (per batch, two passes to bound SBUF):
  pass 0 projects {q0, k, v} block-by-block (512 tokens), applies RoPE,
  transposes V, and runs attention for head 0 fused per block (causal:
  query block tb attends key blocks 0..tb). pass 1 projects {q1,q2,q3}
  and runs attention for heads 1-3. K and transposed V stay resident;
  q lives only per-block. After both passes the per-batch attention
  output [384, 2048] is AllGathered to [3072, 2048] and the output
  projection streams it back.

Device-side layouts (feature dims on SBUF partitions, tokens on free):
 - padded head dim: real dim j at partition (j if j < 48 else j + 16);
   partitions 48..63 / 112..127 are zero pads, so RoPE rotate-half is a
   partition rotation by 64 (two shifted copies).
 - scores computed transposed [keys, q-tokens]; softmax denominator via
   an appended ones-column in transposed V (row 96 of the PV psum).
"""

import numpy as np

import concourse.bass as bass
import concourse.mybir as mybir
import concourse.tile as tile
import concourse.bacc as bacc
from concourse.bass_utils import run_bass_kernel_spmd

N_CORES = 8
B, S, H = 2, 2048, 3072
NH, NKV, HD = 32, 8, 96
T = B * S
ROPE_BASE = 10000.0
KC = H // 128          # 24 contraction chunks
TB = 512               # token block (qkv + attention + o_proj)
NTB = S // TB          # 4 blocks per batch
NKB = S // 128         # 16 key blocks per batch
SCALE = 1.0 / float(np.sqrt(HD))

F32 = mybir.dt.float32
F32R = mybir.dt.float32r

_NC_CACHE = {}


def _round_tf32(x: np.ndarray) -> np.ndarray:
    u = np.ascontiguousarray(x, dtype=np.float32).view(np.uint32)
    r = (u + 0x7FF + ((u >> 12) & 1)) & 0xFFFFF000
    return r.astype(np.uint32).view(np.float32)


def _pad_head(w96: np.ndarray) -> np.ndarray:
    """[96, H] head rows -> [128, H] padded layout."""
    out = np.zeros((128, w96.shape[1]), dtype=w96.dtype)
    out[0:48] = w96[0:48]
    out[64:112] = w96[48:96]
    return out


def _build_nc(repeat: int = 1, sim: bool = False):
    key = ("nc", repeat, sim)
    if key in _NC_CACHE:
        return _NC_CACHE[key]

    nc = bacc.Bacc("TRN2", target_bir_lowering=False, debug=False,
                   num_devices=1 if sim else N_CORES)
    nc._sim_single_core = sim

    x_t = nc.dram_tensor("x_t", [H, T], F32R, kind="ExternalInput").ap()
    w_s = nc.dram_tensor("w_s", [6, 128, KC, 128], F32R,
                         kind="ExternalInput").ap()
    wo_s = nc.dram_tensor("wo_s", [3, 128, KC, 128], F32R,
                          kind="ExternalInput").ap()
    cos_d = nc.dram_tensor("cos_d", [128, T], F32, kind="ExternalInput").ap()
    sin_d = nc.dram_tensor("sin_d", [128, T], F32, kind="ExternalInput").ap()
    mask_d = nc.dram_tensor("mask_d", [128, 896], F32,
                            kind="ExternalInput").ap()
    ident_d = nc.dram_tensor("ident_d", [128, 128], F32,
                             kind="ExternalInput").ap()
    out_d = nc.dram_tensor("out", [384, T], F32, kind="ExternalOutput").ap()

    with tile.TileContext(nc) as tc:
        for rep in range(repeat):
            if rep:
                tc.strict_bb_all_engine_barrier()
            _emit_program(nc, tc, x_t, w_s, wo_s, cos_d, sin_d, mask_d,
                          ident_d, out_d)

    nc.compile()
    _NC_CACHE[key] = nc
    return nc


# fb -> (pass index, w slot, rope kind)
_PASS_FBS = [[0, 4, 5], [1, 2, 3]]    # pass0: q0,k,v ; pass1: q1,q2,q3
_PASS_HEADS = [[0], [1, 2, 3]]


def _emit_program(nc, tc, x_t, w_s, wo_s, cos_d, sin_d, mask_d, ident_d,
                  out_d):
    rg = [list(range(N_CORES))]
    sim1 = getattr(nc, "_sim_single_core", False)
    Exp = mybir.ActivationFunctionType.Exp

    with (
        tc.tile_pool(name="const", bufs=1) as constp,
        tc.tile_pool(name="dram", bufs=1, space="DRAM") as dramp,
    ):
        mask_t = constp.tile([128, 896], F32, name="mask_t")
        nc.sync.dma_start(out=mask_t, in_=mask_d)
        ident_t = constp.tile([128, 128], F32, name="ident_t")
        nc.sync.dma_start(out=ident_t, in_=ident_d)

        ag_in = [dramp.tile([384, S], F32R, name=f"agin{b}", tag=f"agin{b}")
                 for b in range(B)]
        ag_out = [dramp.tile([N_CORES * 384, S], F32R, addr_space="Shared",
                             name=f"agout{b}", tag=f"agout{b}")
                  for b in range(B)]

        with (
            tc.tile_pool(name="pk", bufs=1) as pk,
            tc.tile_pool(name="pa", bufs=1) as pa,
            tc.tile_pool(name="pb", bufs=1) as pb,
            tc.tile_pool(name="psA", bufs=2, space="PSUM") as psA,
            tc.tile_pool(name="psT", bufs=1, space="PSUM") as psT,
            tc.tile_pool(name="psB", bufs=2, space="PSUM") as psB,
        ):
            for b in range(B):
                bs = b * S
                # per-batch K (rope'd) and transposed V, resident via
                # batch-indexed tags (bufs=1 each; freed by slot reuse
                # two batches later -- only 2 batches, so both coexist)
                kf = pk.tile([128, S], F32R, tag=f"kf{b}", name=f"kf{b}")
                vkt = pk.tile([128, NKB * 97], F32R, tag=f"vk{b}",
                              name=f"vk{b}")
                for kb in range(NKB):
                    nc.vector.memset(
                        vkt[:, 97 * kb + 96:97 * kb + 97].bitcast(F32), 1.0)

                for pss in range(2):
                    fbs = _PASS_FBS[pss]
                    heads = _PASS_HEADS[pss]
                    wt = {}
                    for slot, fb in enumerate(fbs):
                        w_tile = pa.tile([128, KC * 128], F32R,
                                         tag=f"w{slot}",
                                         name=f"w_{b}_{pss}_{fb}")
                        nc.sync.dma_start(
                            out=w_tile,
                            in_=w_s[fb].rearrange("p k j -> p (k j)"))
                        wt[fb] = w_tile

                    for tb in range(NTB):
                        ts = slice(bs + tb * TB, bs + (tb + 1) * TB)
                        # x quarter tiles: 6 contraction chunks each
                        xq = []
                        for q in range(4):
                            xt = pa.tile([128, 6 * TB], F32R, tag=f"x{q}",
                                         name=f"x_{b}_{pss}_{tb}_{q}")
                            nc.sync.dma_start(
                                out=xt.rearrange("p (k t) -> p k t", k=6),
                                in_=x_t[768 * q:768 * (q + 1), ts].rearrange(
                                    "(k p) t -> p k t", p=128))
                            xq.append(xt)
                        cos_tb = pa.tile([128, TB], F32, tag="cos", bufs=2,
                                         name=f"cos_{b}_{pss}_{tb}")
                        sin_tb = pa.tile([128, TB], F32, tag="sin", bufs=2,
                                         name=f"sin_{b}_{pss}_{tb}")
                        nc.sync.dma_start(out=cos_tb, in_=cos_d[:, ts])
                        nc.sync.dma_start(out=sin_tb, in_=sin_d[:, ts])

                        q_rope = {}
                        for fb in fbs:
                            ps = psA.tile([128, TB], F32, tag="qkv",
                                          name=f"ps_{b}_{pss}_{tb}_{fb}")
                            for k in range(KC):
                                nc.tensor.matmul(
                                    ps, wt[fb][:, k * 128:(k + 1) * 128],
                                    xq[k // 6][:, (k % 6) * TB:
                                               (k % 6 + 1) * TB],
                                    start=(k == 0), stop=(k == KC - 1))
                            if fb == 5:
                                # V: transpose to [keys, d] layout
                                vst = pa.tile([128, TB], F32, tag="vst",
                                              bufs=2, name=f"vst_{b}_{tb}")
                                nc.scalar.copy(vst, ps)
                                for i in range(TB // 128):
                                    kb = tb * (TB // 128) + i
                                    pt = psT.tile([128, 128], F32, tag="tr",
                                                  name=f"pt_{b}_{tb}_{i}")
                                    nc.tensor.transpose(
                                        pt, vst[:, i * 128:(i + 1) * 128],
                                        ident_t)
                                    # cols {0:48, 64:112} -> vkt 96 cols
                                    src = pt.rearrange(
                                        "p (a c) -> p a c", a=2)[:, :, 0:48]
                                    dst = vkt[:, 97 * kb:97 * kb + 96] \
                                        .rearrange("p (a c) -> p a c", a=2)
                                    nc.vector.tensor_copy(dst, src)
                            else:
                                # RoPE from psum
                                if fb == 4:
                                    dst = kf[:, tb * TB:(tb + 1) * TB]
                                else:
                                    qt = pa.tile([128, TB], F32R,
                                                 tag=f"q{fb}", bufs=2,
                                                 name=f"q_{b}_{pss}_{tb}_{fb}")
                                    q_rope[fb] = qt
                                    dst = qt
                                rot = pa.tile([128, TB], F32, tag="rot",
                                              bufs=2,
                                              name=f"rot_{b}_{pss}_{tb}_{fb}")
                                nc.vector.tensor_copy(rot[0:64, :],
                                                      ps[64:128, :])
                                nc.vector.tensor_copy(rot[64:128, :],
                                                      ps[0:64, :])
                                nc.vector.tensor_mul(rot, rot, sin_tb)
                                cq = pa.tile([128, TB], F32, tag="cq",
                                             bufs=2,
                                             name=f"cq_{b}_{pss}_{tb}_{fb}")
                                nc.vector.tensor_mul(cq, ps, cos_tb)
                                nc.vector.tensor_add(dst, rot, cq)

                        # fused attention for this pass's heads, qb = tb
                        for h in heads:
                            qt = q_rope[h]
                            pv = psB.tile([128, TB], F32, tag="pv",
                                          name=f"pv_{b}_{h}_{tb}")
                            nkb = (tb + 1) * (TB // 128)
                            diag0 = tb * (TB // 128)
                            for kb in range(nkb):
                                sc = psB.tile([128, TB], F32, tag="sc",
                                              bufs=3,
                                              name=f"sc_{b}_{h}_{tb}_{kb}")
                                nc.tensor.matmul(
                                    sc, kf[:, kb * 128:(kb + 1) * 128], qt,
                                    start=True, stop=True)
                                pr = pb.tile([128, TB], F32R, tag="pr",
                                             bufs=3,
                                             name=f"pr_{b}_{h}_{tb}_{kb}")
                                if kb >= diag0:
                                    prf = pb.tile(
                                        [128, TB], F32, tag="prf", bufs=2,
                                        name=f"prf_{b}_{h}_{tb}_{kb}")
                                    nc.scalar.activation(prf, sc, Exp,
                                                         scale=SCALE)
                                    off = (kb - diag0) * 128
                                    nc.gpsimd.tensor_mul(
                                        pr, prf,
                                        mask_t[:, 384 - off:896 - off])
                                else:
                                    nc.scalar.activation(pr, sc, Exp,
                                                         scale=SCALE)
                                nc.tensor.matmul(
                                    pv[0:97, :],
                                    vkt[:, 97 * kb:97 * kb + 97], pr,
                                    start=(kb == 0), stop=(kb == nkb - 1))
                            # normalize: attn = pv[0:96] / pv[96]
                            l_r = pb.tile([1, TB], F32, tag="lr", bufs=2,
                                          name=f"lr_{b}_{h}_{tb}")
                            nc.vector.reciprocal(l_r, pv[96:97, :])
                            nt = pb.tile([1, TB], F32, tag="nt", bufs=2,
                                         name=f"nt_{b}_{h}_{tb}")
                            nc.vector.tensor_mul(nt, pv[96:97, :], l_r)
                            nc.vector.tensor_scalar(
                                nt, nt, -1.0, 2.0,
                                op0=mybir.AluOpType.mult,
                                op1=mybir.AluOpType.add)
                            nc.vector.tensor_mul(l_r, l_r, nt)
                            bc = pb.tile([96, TB], F32, tag="bc", bufs=2,
                                         name=f"bc_{b}_{h}_{tb}")
                            nc.gpsimd.partition_broadcast(bc, l_r)
                            asb = pb.tile([96, TB], F32R, tag="asb", bufs=3,
                                          name=f"asb_{b}_{h}_{tb}")
                            nc.vector.tensor_mul(asb, pv[0:96, :], bc)
                            nc.sync.dma_start(
                                out=ag_in[b][96 * h:96 * (h + 1),
                                             tb * TB:(tb + 1) * TB],
                                in_=asb)

                if sim1:
                    nc.sync.dma_start(out=ag_out[b][0:384, :], in_=ag_in[b])
                else:
                    nc.gpsimd.collective_compute(
                        "AllGather", mybir.AluOpType.bypass,
                        replica_groups=rg,
                        ins=[ag_in[b].opt()], outs=[ag_out[b].opt()])

        # ---------------- output projection --------------------------
        with (
            tc.tile_pool(name="pc", bufs=1) as pc,
            tc.tile_pool(name="psC", bufs=2, space="PSUM") as psC,
        ):
            wo = []
            for jb in range(3):
                wo_tile = pc.tile([128, KC * 128], F32R, tag=f"wo{jb}",
                                  name=f"wo_{jb}")
                nc.sync.dma_start(
                    out=wo_tile, in_=wo_s[jb].rearrange("p k j -> p (k j)"))
                wo.append(wo_tile)
            for b in range(B):
                for tb in range(NTB):
                    cs = slice(tb * TB, (tb + 1) * TB)
                    ats = []
                    for q in range(4):
                        at = pc.tile([128, 6 * TB], F32R, tag=f"at{q}",
                                     bufs=2, name=f"at_{b}_{tb}_{q}")
                        nc.sync.dma_start(
                            out=at.rearrange("p (k t) -> p k t", k=6),
                            in_=ag_out[b][768 * q:768 * (q + 1), cs]
                            .rearrange("(k p) t -> p k t", p=128))
                        ats.append(at)
                    for jb in range(3):
                        ps = psC.tile([128, TB], F32, tag="op",
                                      name=f"ops_{b}_{tb}_{jb}")
                        for k in range(KC):
                            nc.tensor.matmul(
                                ps, wo[jb][:, k * 128:(k + 1) * 128],
                                ats[k // 6][:, (k % 6) * TB:(k % 6 + 1) * TB],
                                start=(k == 0), stop=(k == KC - 1))
                        osb = pc.tile([128, TB], F32, tag="ost", bufs=3,
                                      name=f"osb_{b}_{tb}_{jb}")
                        nc.vector.tensor_copy(osb, ps)
                        nc.sync.dma_start(
                            out=out_d[jb * 128:(jb + 1) * 128,
                                      b * S + tb * TB:b * S + (tb + 1) * TB],
                            in_=osb)


def _prep_inputs(hidden_states, position_ids, w_qkv, w_o):
    hidden_states = np.asarray(hidden_states, dtype=np.float32)
    position_ids = np.asarray(position_ids)
    w_qkv = np.asarray(w_qkv, dtype=np.float32)
    w_o = np.asarray(w_o, dtype=np.float32)

    x_t = _round_tf32(hidden_states.reshape(T, H).T)

    pos = position_ids.reshape(T).astype(np.float64)
    inv_freq = 1.0 / (ROPE_BASE ** (np.arange(0, HD, 2, dtype=np.float64)
                                    / HD))  # [48]
    freqs = pos[None, :] * inv_freq[:, None]          # [48, T]
    cos48, sin48 = np.cos(freqs), np.sin(freqs)
    cos_d = np.zeros((128, T), dtype=np.float32)
    sin_d = np.zeros((128, T), dtype=np.float32)
    cos_d[0:48] = cos48
    cos_d[64:112] = cos48
    sin_d[0:48] = -sin48
    sin_d[64:112] = sin48

    cc = np.arange(896)[None, :]
    kk = np.arange(128)[:, None]
    mask_ext = (cc >= kk + 384).astype(np.float32)

    ident = np.eye(128, dtype=np.float32)

    common = {"x_t": x_t, "cos_d": cos_d, "sin_d": sin_d,
              "mask_d": mask_ext, "ident_d": ident}

    in_maps = []
    for r in range(N_CORES):
        w_big = np.zeros((6, 128, H), dtype=np.float32)
        for fb in range(4):
            qh = 4 * r + fb
            w_big[fb] = _pad_head(w_qkv[96 * qh:96 * (qh + 1), :])
        w_big[4] = _pad_head(w_qkv[NH * HD + 96 * r: NH * HD + 96 * (r + 1)])
        w_big[5] = _pad_head(
            w_qkv[NH * HD + NKV * HD + 96 * r:
                  NH * HD + NKV * HD + 96 * (r + 1)])
        # [6, 128(j), 24(k), 128(p)] -> [6, 128(p), 24(k), 128(j)]
        w_dev = np.ascontiguousarray(
            w_big.reshape(6, 128, KC, 128).transpose(0, 3, 2, 1))
        w_dev = _round_tf32(w_dev)

        wo_shard = w_o[384 * r:384 * (r + 1), :]   # [384, 3072]
        wo_dev = np.ascontiguousarray(
            wo_shard.reshape(3, 128, KC, 128).transpose(0, 3, 2, 1))
        wo_dev = _round_tf32(wo_dev)

        in_maps.append({**common, "w_s": w_dev, "wo_s": wo_dev})
    return in_maps


def kernel(hidden_states, position_ids, w_qkv, w_o):
    in_maps = _prep_inputs(hidden_states, position_ids, w_qkv, w_o)
    nc = _build_nc()
    res = run_bass_kernel_spmd(nc, in_maps, core_ids=list(range(N_CORES)))
    big = np.concatenate([res.results[r]["out"] for r in range(N_CORES)],
                         axis=0)                      # [3072, 4096]
    out = big.reshape(H, B, S).transpose(1, 2, 0)     # [B, S, H]
    return np.ascontiguousarray(out, dtype=np.float32)
